# revision 2
# baseline (speedup 1.0000x reference)
"""AlexNet-style CNN forward pass on 8 Trainium2 NeuronCores (bf16).

Strategy:
  - Convs data-parallel: batch 256 -> 32 per core, channels on partitions,
    conv = sum of shifted matmuls over kernel offsets (weights replicated).
    All matmuls bf16 (3.3x faster than fp32r on HW: fp32 streams 4 cyc/elem
    and disables fast-weight-load).
  - conv1 uses host-packed im2col rows (3 dy-offsets x 11 dx x 3 ch + ones
    row for fused bias -> K=100); output channels duplicated to M=128 so
    the x+1-shifted copy of conv2's input is written by partition-local
    evict ops instead of a 45K-descriptor DMA.
  - conv2: K=128 = 2 dx-offsets x 64 ch via the shifted duplicate; dx=6
    column handled by pairing two dy rows on opposite 64-partition halves
    (concurrent row-group matmuls).  mc=1 output block computes ch128:192
    twice (M=128) so conv3's duplicated-K buffer is written directly.
  - conv3: ch0:128 aligned (X3a) + ch128:192 duplicated on both halves
    (X3b); 25 full-K matmuls + 13 paired half-K matmuls per output tile.
  - FC layers model-parallel: each core owns 512 rows of fc1/fc2 and 512
    K-columns of fc3; activations AllGathered between layers (bf16, 2
    chunks overlapped with compute), fc3 partials AllReduced.
"""

import numpy as np
import ml_dtypes

import concourse.bass as bass
import concourse.mybir as mybir
import concourse.tile as tile
from concourse import bacc
from concourse.bass_utils import run_bass_kernel_spmd

N_CORES = 8
B = 256
BC = B // N_CORES  # 32 images per core

F32 = mybir.dt.float32
BF16 = mybir.dt.bfloat16
RELU = mybir.ActivationFunctionType.Relu
IDENT = mybir.ActivationFunctionType.Identity
BF = ml_dtypes.bfloat16


def _emit(nc, tc, t, yout):
    sync = nc.sync
    act = nc.scalar
    dve = nc.vector
    pool_e = nc.gpsimd

    psum = tc.alloc_tile_pool(name="psum", bufs=6, space="PSUM")
    scr = tc.alloc_tile_pool(name="scr", bufs=1, side="left")
    dram = tc.alloc_tile_pool(name="dram", bufs=1, space="DRAM")

    # ---------------- left-stack pools: conv1/conv2 era ------------------
    p_w12 = tc.alloc_tile_pool(name="p_w12", bufs=1, side="left")
    p_x2 = tc.alloc_tile_pool(name="p_x2", bufs=1, side="left")
    p_x13 = tc.alloc_tile_pool(name="p_x13", bufs=2, side="left")

    lw1 = p_w12.tile([100, 4 * 128], BF16)
    sync.dma_start(lw1[:], t["lw1"][:])
    lw2 = p_w12.tile([128, 7 * 3 * 256], BF16)
    sync.dma_start(lw2[:], t["lw2"][:])
    lw2d6 = p_w12.tile([128, 4 * 256], BF16)
    sync.dma_start(lw2d6[:], t["lw2d6"][:])
    lb2 = p_w12.tile([128, 2], F32)
    sync.dma_start(lb2[:], t["lb2"][:])

    # conv2 input: [128, BC, 22, 23]; rows 0:64 ch c at x, rows 64:128 at x+1
    X2s = p_x2.tile([128, BC * 22 * 23], BF16)
    pool_e.memset(X2s[:].bitcast(F32), 0.0)

    def x2v(p0, p1, b0, nb, y0, ny, x0, nx):
        return X2s[p0:p1].rearrange("p (b y x) -> p b y x", b=BC, y=22, x=23)[
            :, b0:b0 + nb, y0:y0 + ny, x0:x0 + nx]

    # ---------------- conv1 + pool1 ----------------
    _sid = nc.enter_named_scope("L1_conv1", False)[0]
    for bg in range(8):  # groups of 4 images
        xt = p_x13.tile([100, 4 * 41 * 32], BF16, tag="x13")
        sync.dma_start(xt[:], t["x13"][bg])
        xtv = xt.rearrange("k (b y x) -> k b y x", b=4, y=41, x=32)
        for bl in range(4):
            b = bg * 4 + bl
            for h in range(2):  # vertical half of the 32x32 output
                ps = psum.tile([128, 512], F32, tag="ps")
                for pi in range(4):
                    nc.tensor.matmul(
                        ps[:],
                        lw1[:, pi * 128:(pi + 1) * 128],
                        xtv[:, bl, h * 16 + 3 * pi:h * 16 + 3 * pi + 16, :],
                        start=(pi == 0), stop=(pi == 3),
                    )
                psv = ps.rearrange("m (y x) -> m y x", y=16, x=32)
                m1 = scr.tile([128, 256], BF16, tag="m1", bufs=3)
                m2 = scr.tile([128, 128], BF16, tag="m2", bufs=3)
                dve.tensor_max(m1.rearrange("m (y x) -> m y x", y=16, x=16),
                               psv[:, :, 0::2], psv[:, :, 1::2])
                m1v = m1.rearrange("m (y x) -> m y x", y=16, x=16)
                pool_e.tensor_max(m2.rearrange("m (y x) -> m y x", y=8, x=16),
                                  m1v[:, 0::2, :], m1v[:, 1::2, :])
                m2v = m2.rearrange("m (y x) -> m y x", y=8, x=16)
                y0 = h * 8 + 3
                # relu (bias came in via the ones-row); write lo half at x,
                # hi half at x-1 (hi partitions hold the x+1-shifted copy)
                act.activation(x2v(0, 64, b, 1, y0, 8, 3, 16)[:, 0],
                               m2v[0:64], RELU)
                pool_e.tensor_scalar_max(x2v(64, 128, b, 1, y0, 8, 2, 16)[:, 0],
                                         m2v[64:128], 0.0)
    p_x13.release()
    nc.leave_named_scope("L1_conv1", _sid, False)

    # conv3 weights (prefetch during conv2) + conv3 input buffers
    p_w3 = tc.alloc_tile_pool(name="p_w3", bufs=1, side="right")
    p_x3 = tc.alloc_tile_pool(name="p_x3", bufs=1, side="right")
    lw3a = p_w3.tile([128, 25 * 384], BF16)
    sync.dma_start(lw3a[:], t["lw3a"][:])
    lw3b = p_w3.tile([128, 13 * 384], BF16)
    sync.dma_start(lw3b[:], t["lw3b"][:])
    lb3 = p_w3.tile([128, 3], F32)
    sync.dma_start(lb3[:], t["lb3"][:])
    X3a = p_x3.tile([128, BC * 12 * 12], BF16)
    X3b = p_x3.tile([128, BC * 12 * 12], BF16)
    pool_e.memset(X3a[:].bitcast(F32), 0.0)
    pool_e.memset(X3b[:].bitcast(F32), 0.0)

    def x3v(xab, p0, p1, b0, nb, y0, ny, x0, nx):
        return xab[p0:p1].rearrange("p (b y x) -> p b y x", b=BC, y=12, x=12)[
            :, b0:b0 + nb, y0:y0 + ny, x0:x0 + nx]

    # ---------------- conv2 + pool2 ----------------
    _sid = nc.enter_named_scope("L2_conv2", False)[0]
    lw2v = lw2.rearrange("k (d j m) -> k d j m", d=7, j=3, m=256)
    lw2d6v = lw2d6.rearrange("k (a m) -> k a m", a=4)
    for nt in range(16):  # pairs of images
        for mc in range(2):
            ps = psum.tile([128, 512], F32, tag="ps")
            first = True
            for dy in range(7):
                for j in range(3):
                    nc.tensor.matmul(
                        ps[:],
                        lw2v[:, dy, j, mc * 128:mc * 128 + 128],
                        x2v(0, 128, nt * 2, 2, dy, 16, 2 * j, 16),
                        start=first, stop=False,
                    )
                    first = False
            # dx=6 column: pair dy rows on opposite partition halves
            for a in range(3):
                nc.tensor.matmul(
                    ps[:], lw2d6v[0:64, a, mc * 128:mc * 128 + 128],
                    x2v(0, 64, nt * 2, 2, 2 * a, 16, 6, 16),
                    start=False, stop=False)
                nc.tensor.matmul(
                    ps[:], lw2d6v[64:128, a, mc * 128:mc * 128 + 128],
                    x2v(64, 128, nt * 2, 2, 2 * a + 1, 16, 5, 16),
                    start=False, stop=False)
            nc.tensor.matmul(
                ps[:], lw2d6v[0:64, 3, mc * 128:mc * 128 + 128],
                x2v(0, 64, nt * 2, 2, 6, 16, 6, 16),
                start=False, stop=True)
            # pool 2x2 + relu + bias -> X3a (mc=0) / X3b (mc=1, dup'd H2)
            psv = ps.rearrange("m (b y x) -> m b y x", b=2, y=16, x=16)
            m1 = scr.tile([128, 256], BF16, tag="m1", bufs=3)
            m2 = scr.tile([128, 128], BF16, tag="m2", bufs=3)
            dve.tensor_max(m1.rearrange("m (b y x) -> m b y x", b=2, y=16, x=8),
                           psv[:, :, :, 0::2], psv[:, :, :, 1::2])
            m1v = m1.rearrange("m (b y x) -> m b y x", b=2, y=16, x=8)
            pool_e.tensor_max(m2.rearrange("m (b y x) -> m b y x", b=2, y=8, x=8),
                              m1v[:, :, 0::2, :], m1v[:, :, 1::2, :])
            m2v = m2.rearrange("m (b y x) -> m b y x", b=2, y=8, x=8)
            dst = X3a if mc == 0 else X3b
            act.activation(x3v(dst, 0, 128, nt * 2, 2, 2, 8, 2, 8),
                           m2v[:], RELU, bias=lb2[:, mc:mc + 1])
    nc.leave_named_scope("L2_conv2", _sid, False)
    p_x2.release()
    p_w12.release()

    # conv4/5 weights (prefetch during conv3) + conv4 input buffers
    p_w45 = tc.alloc_tile_pool(name="p_w45", bufs=1, side="left")
    p_x4 = tc.alloc_tile_pool(name="p_x4", bufs=1, side="left")
    lw4 = p_w45.tile([128, 27 * 256], BF16)
    sync.dma_start(lw4[:], t["lw4"][:])
    lb4 = p_w45.tile([128, 2], F32)
    sync.dma_start(lb4[:], t["lb4"][:])
    lw5 = p_w45.tile([128, 18 * 256], BF16)
    sync.dma_start(lw5[:], t["lw5"][:])
    lb5 = p_w45.tile([128, 2], F32)
    sync.dma_start(lb5[:], t["lb5"][:])
    X4 = []
    for i in range(3):
        X4.append(p_x4.tile([128, BC * 10 * 10], BF16, name=f"X4_{i}"))
        pool_e.memset(X4[i][:].bitcast(F32), 0.0)

    def xv10(xab, p0, p1, b0, nb, y0, ny, x0, nx):
        return xab[p0:p1].rearrange("p (b y x) -> p b y x", b=BC, y=10, x=10)[
            :, b0:b0 + nb, y0:y0 + ny, x0:x0 + nx]

    _sid = nc.enter_named_scope("L3_conv3", False)[0]
    # ---------------- conv3 ----------------
    lw3av = lw3a.rearrange("k (o m) -> k o m", o=25)
    lw3bv = lw3b.rearrange("k (s m) -> k s m", s=13)
    for nt in range(4):  # 8 images
        for mc in range(3):
            ps = psum.tile([128, 512], F32, tag="ps")
            for o in range(25):
                dy, dx = o // 5, o % 5
                nc.tensor.matmul(
                    ps[:], lw3av[:, o, mc * 128:mc * 128 + 128],
                    x3v(X3a, 0, 128, nt * 8, 8, dy, 8, dx, 8),
                    start=(o == 0), stop=False)
            for s in range(12):
                o1, o2 = 2 * s, 2 * s + 1
                nc.tensor.matmul(
                    ps[:], lw3bv[0:64, s, mc * 128:mc * 128 + 128],
                    x3v(X3b, 0, 64, nt * 8, 8, o1 // 5, 8, o1 % 5, 8),
                    start=False, stop=False)
                nc.tensor.matmul(
                    ps[:], lw3bv[64:128, s, mc * 128:mc * 128 + 128],
                    x3v(X3b, 64, 128, nt * 8, 8, o2 // 5, 8, o2 % 5, 8),
                    start=False, stop=False)
            nc.tensor.matmul(
                ps[:], lw3bv[0:64, 12, mc * 128:mc * 128 + 128],
                x3v(X3b, 0, 64, nt * 8, 8, 4, 8, 4, 8),
                start=False, stop=True)
            act.activation(
                xv10(X4[mc], 0, 128, nt * 8, 8, 1, 8, 1, 8),
                ps.rearrange("m (b y x) -> m b y x", b=8, y=8, x=8),
                RELU, bias=lb3[:, mc:mc + 1])
    nc.leave_named_scope("L3_conv3", _sid, False)
    p_x3.release()
    p_w3.release()

    # fc1 weights (prefetch during conv4) + conv5 input buffers
    p_fw1 = tc.alloc_tile_pool(name="p_fw1", bufs=1, side="right")
    p_x5 = tc.alloc_tile_pool(name="p_x5", bufs=1, side="right")
    fw1 = p_fw1.tile([128, 32 * 512], BF16)
    sync.dma_start(fw1[:], t["fw1s"][:])
    fb1 = p_fw1.tile([128, 4], F32)
    sync.dma_start(fb1[:], t["fb1s"][:])
    X5 = []
    for i in range(2):
        X5.append(p_x5.tile([128, BC * 10 * 10], BF16, name=f"X5_{i}"))
        pool_e.memset(X5[i][:].bitcast(F32), 0.0)

    _sid = nc.enter_named_scope("L4_conv4", False)[0]
    # ---------------- conv4 ----------------
    lw4v = lw4.rearrange("k (o m) -> k o m", o=27)
    for nt in range(4):
        for mc in range(2):
            ps = psum.tile([128, 512], F32, tag="ps")
            first = True
            for dy in range(3):
                for dx in range(3):
                    for kc in range(3):
                        o = (dy * 3 + dx) * 3 + kc
                        nc.tensor.matmul(
                            ps[:],
                            lw4v[:, o, mc * 128:mc * 128 + 128],
                            xv10(X4[kc], 0, 128, nt * 8, 8, dy, 8, dx, 8),
                            start=first, stop=(o == 26),
                        )
                        first = False
            act.activation(
                xv10(X5[mc], 0, 128, nt * 8, 8, 1, 8, 1, 8),
                ps.rearrange("m (b y x) -> m b y x", b=8, y=8, x=8),
                RELU, bias=lb4[:, mc:mc + 1])
    nc.leave_named_scope("L4_conv4", _sid, False)
    p_x4.release()

    # fc2/fc3 weights (prefetch during conv5)
    p_fw2 = tc.alloc_tile_pool(name="p_fw2", bufs=1, side="left")
    fw2 = p_fw2.tile([128, 32 * 512], BF16)
    sync.dma_start(fw2[:], t["fw2s"][:])
    fb2 = p_fw2.tile([128, 4], F32)
    sync.dma_start(fb2[:], t["fb2s"][:])
    fw3 = p_fw2.tile([128, 4 * 100], BF16)
    sync.dma_start(fw3[:], t["fw3s"][:])
    fb3 = p_fw2.tile([100, 1], F32)
    sync.dma_start(fb3[:], t["fb3s"][:])

    # pool5 -> DRAM staging (2 chunks of 16 images for the AllGather)
    cin5 = dram.tile([2, 2, 128, 256], BF16)  # [chunk, mc, c, 16img*16yx]
    g1 = dram.tile([2, N_CORES, 2, 128, 256], BF16)

    _sid = nc.enter_named_scope("L5_conv5", False)[0]
    # ---------------- conv5 + pool5 + chunked AllGather ----------------
    lw5v = lw5.rearrange("k (o m) -> k o m", o=18)
    for nt in range(4):
        for mc in range(2):
            ps = psum.tile([128, 512], F32, tag="ps")
            first = True
            for dy in range(3):
                for dx in range(3):
                    for kc in range(2):
                        o = (dy * 3 + dx) * 2 + kc
                        nc.tensor.matmul(
                            ps[:],
                            lw5v[:, o, mc * 128:mc * 128 + 128],
                            xv10(X5[kc], 0, 128, nt * 8, 8, dy, 8, dx, 8),
                            start=first, stop=(o == 17),
                        )
                        first = False
            psv = ps.rearrange("m (b y x) -> m b y x", b=8, y=8, x=8)
            m1 = scr.tile([128, 256], BF16, tag="m1", bufs=3)
            m2 = scr.tile([128, 128], BF16, tag="m2", bufs=3)
            dve.tensor_max(m1.rearrange("m (b y x) -> m b y x", b=8, y=8, x=4),
                           psv[:, :, :, 0::2], psv[:, :, :, 1::2])
            m1v = m1.rearrange("m (b y x) -> m b y x", b=8, y=8, x=4)
            pool_e.tensor_max(m2.rearrange("m (b y x) -> m b y x", b=8, y=4, x=4),
                              m1v[:, :, 0::2, :], m1v[:, :, 1::2, :])
            p5t = scr.tile([128, 128], BF16, tag="p5t", bufs=2)
            act.activation(p5t[:], m2[:], RELU, bias=lb5[:, mc:mc + 1])
            sync.dma_start(
                cin5[nt // 2, mc, :, (nt % 2) * 128:(nt % 2) * 128 + 128],
                p5t[:])
        if nt % 2 == 1:
            h = nt // 2
            pool_e.collective_compute(
                "AllGather", mybir.AluOpType.bypass,
                replica_groups=[list(range(N_CORES))],
                ins=[cin5[h].opt()], outs=[g1[h].opt()])
    nc.leave_named_scope("L5_conv5", _sid, False)
    p_x5.release()
    p_w45.release()

    _sid = nc.enter_named_scope("G1_gather", False)[0]
    # ---------------- assemble fc1 input [c, (r, b32, yx16)] -------------
    p_h1 = tc.alloc_tile_pool(name="p_h1", bufs=1, side="right")
    H1 = [p_h1.tile([128, N_CORES * BC * 16], BF16, name=f"H1_{i}")
          for i in range(2)]
    for h in range(2):
        for cc in range(2):
            sync.dma_start(
                H1[cc].rearrange("c (r b y) -> c r b y", r=N_CORES, b=BC)[
                    :, :, h * 16:(h + 1) * 16, :]
                .rearrange("c r b y -> c r (b y)"),
                g1[h, :, cc].rearrange("r c f -> c r f"))
    nc.leave_named_scope("G1_gather", _sid, False)

    _sid = nc.enter_named_scope("F1_fc1", False)[0]
    # ---------------- fc1 (model-parallel over 512 outputs) --------------
    p_f1 = tc.alloc_tile_pool(name="p_f1", bufs=1, side="left")
    F1 = p_f1.tile([128, 4 * B], BF16)
    cin6 = dram.tile([2, 128, 512], BF16)
    g2 = dram.tile([2, N_CORES, 128, 512], BF16)
    fw1v = fw1.rearrange("k (y c m) -> k y c m", y=16, c=2, m=512)
    for mc in range(4):
        ps = psum.tile([128, B], F32, tag="ps")
        first = True
        for yx in range(16):
            for cc in range(2):
                rhs = H1[cc].rearrange("c (r b y) -> c y r b", r=N_CORES,
                                       b=BC, y=16)
                nc.tensor.matmul(
                    ps[:],
                    fw1v[:, yx, cc, mc * 128:mc * 128 + 128],
                    rhs[:, yx],
                    start=first, stop=(yx == 15 and cc == 1))
                first = False
        act.activation(F1[:, mc * B:(mc + 1) * B], ps[:], RELU,
                       bias=fb1[:, mc:mc + 1])
        if mc % 2 == 1:
            p = mc // 2
            sync.dma_start(cin6[p], F1[:, p * 512:(p + 1) * 512])
            pool_e.collective_compute(
                "AllGather", mybir.AluOpType.bypass,
                replica_groups=[list(range(N_CORES))],
                ins=[cin6[p].opt()], outs=[g2[p].opt()])
    p_h1.release()
    p_fw1.release()
    nc.leave_named_scope("F1_fc1", _sid, False)

    _sid = nc.enter_named_scope("G2_gather", False)[0]
    # ---------------- assemble fc2 input [c, (r, mc4, img... )] ----------
    p_h2 = tc.alloc_tile_pool(name="p_h2", bufs=1, side="right")
    H2 = p_h2.tile([128, N_CORES * 4 * B], BF16)
    for p in range(2):
        sync.dma_start(
            H2.rearrange("c (r q f) -> c r q f", r=N_CORES, q=2)[:, :, p, :]
            .rearrange("c r f -> c r f"),
            g2[p].rearrange("r c f -> c r f"))
    nc.leave_named_scope("G2_gather", _sid, False)

    _sid = nc.enter_named_scope("F2_fc2", False)[0]
    # ---------------- fc2 ----------------
    p_f2 = tc.alloc_tile_pool(name="p_f2", bufs=1, side="left")
    F2 = p_f2.tile([128, 4 * B], BF16)
    fw2v = fw2.rearrange("k (a m) -> k a m", a=32)
    for mc in range(4):
        ps = psum.tile([128, B], F32, tag="ps")
        for kc in range(32):
            nc.tensor.matmul(
                ps[:], fw2v[:, kc, mc * 128:mc * 128 + 128],
                H2[:, kc * B:(kc + 1) * B],
                start=(kc == 0), stop=(kc == 31))
        act.activation(F2[:, mc * B:(mc + 1) * B], ps[:], RELU,
                       bias=fb2[:, mc:mc + 1])
    p_h2.release()
    nc.leave_named_scope("F2_fc2", _sid, False)

    _sid = nc.enter_named_scope("F3_fc3", False)[0]
    # ---------------- fc3 (partial over this core's 512 K) + AllReduce ---
    fw3v = fw3.rearrange("k (a m) -> k a m", a=4)
    ps = psum.tile([128, B], F32, tag="ps")
    for kc in range(4):
        nc.tensor.matmul(
            ps[0:100, :], fw3v[:, kc, :], F2[:, kc * B:(kc + 1) * B],
            start=(kc == 0), stop=(kc == 3))
    s3 = scr.tile([128, 512], F32, tag="ev", bufs=2)
    s3v = s3[0:100, 0:B]
    act.activation(s3v, ps[0:100, :], IDENT, bias=fb3[:])  # + fb3/8
    cin7 = dram.tile([100, B], F32)
    sync.dma_start(cin7[:], s3v)
    g3 = dram.tile([100, B], F32)
    pool_e.collective_compute(
        "AllReduce", mybir.AluOpType.add,
        replica_groups=[list(range(N_CORES))],
        ins=[cin7.opt()], outs=[g3.opt()])
    sync.dma_start(yout[:], g3[:])
    nc.leave_named_scope("F3_fc3", _sid, False)
    p_f2.release()
    p_f1.release()
    p_fw2.release()

    scr.release()
    dram.release()
    psum.release()


# ---------------------------------------------------------------------------
# host-side input prep (numpy; all weight arrays already in SBUF layout)
# ---------------------------------------------------------------------------

def _prep_shared(w1, b1, w2, b2, w3, b3, w4, b4, w5, b5):
    f = np.float32
    # conv1: rows r = dyo*33 + dx*3 + c, row 99 = bias(ones); M=128 = dup'd 64
    lw1 = np.zeros((100, 4 * 128), f)
    for p in range(4):
        for dyo in range(3):
            dy = 3 * p + dyo
            if dy > 10:
                continue
            for dx in range(11):
                for c in range(3):
                    r = dyo * 33 + dx * 3 + c
                    lw1[r, p * 128:p * 128 + 64] = w1[:, c, dy, dx]
                    lw1[r, p * 128 + 64:p * 128 + 128] = w1[:, c, dy, dx]
    lw1[99, 0:64] = b1
    lw1[99, 64:128] = b1

    def mexp(wt):  # [192 out, 64 in] -> [64, 256] with H2 duplicated
        return np.concatenate([wt[0:128].T, wt[128:192].T, wt[128:192].T],
                              axis=1)

    # conv2: [128, (dy7, j3, m256)]: rows s*64+c = ch c at dx=2j+s
    lw2 = np.zeros((128, 7 * 3 * 256), f)
    for dy in range(7):
        for j in range(3):
            blk = dy * 3 + j
            for s in range(2):
                lw2[s * 64:(s + 1) * 64, blk * 256:(blk + 1) * 256] = \
                    mexp(w2[:, :, dy, 2 * j + s])
    # dx=6: slot a<3 pairs dy=2a (lo rows) with dy=2a+1 (hi rows); slot 3 dy=6
    lw2d6 = np.zeros((128, 4 * 256), f)
    for a in range(3):
        lw2d6[0:64, a * 256:(a + 1) * 256] = mexp(w2[:, :, 2 * a, 6])
        lw2d6[64:128, a * 256:(a + 1) * 256] = mexp(w2[:, :, 2 * a + 1, 6])
    lw2d6[0:64, 3 * 256:4 * 256] = mexp(w2[:, :, 6, 6])
    lb2 = np.zeros((128, 2), f)
    lb2[:, 0] = b2[0:128]
    lb2[:, 1] = np.concatenate([b2[128:192], b2[128:192]])

    # conv3: A tiles [128, (o25, m384)] ch0:128; B tiles [128, (s13, m384)]
    lw3a = np.zeros((128, 25 * 384), f)
    for o in range(25):
        dy, dx = o // 5, o % 5
        lw3a[:, o * 384:(o + 1) * 384] = w3[:, 0:128, dy, dx].T
    lw3b = np.zeros((128, 13 * 384), f)
    for s in range(12):
        o1, o2 = 2 * s, 2 * s + 1
        lw3b[0:64, s * 384:(s + 1) * 384] = w3[:, 128:192, o1 // 5, o1 % 5].T
        lw3b[64:128, s * 384:(s + 1) * 384] = w3[:, 128:192, o2 // 5, o2 % 5].T
    lw3b[0:64, 12 * 384:13 * 384] = w3[:, 128:192, 4, 4].T
    lb3 = np.zeros((128, 3), f)
    lb3[:, 0] = b3[0:128]; lb3[:, 1] = b3[128:256]; lb3[:, 2] = b3[256:384]

    # conv4 / conv5: [128, (o, m)] with o = (dy*3+dx)*nkc + kc
    lw4 = np.zeros((128, 27 * 256), f)
    for dy in range(3):
        for dx in range(3):
            for kc in range(3):
                o = (dy * 3 + dx) * 3 + kc
                lw4[:, o * 256:(o + 1) * 256] = w4[:, kc * 128:(kc + 1) * 128, dy, dx].T
    lb4 = np.stack([b4[0:128], b4[128:256]], axis=1).astype(f)
    lw5 = np.zeros((128, 18 * 256), f)
    for dy in range(3):
        for dx in range(3):
            for kc in range(2):
                o = (dy * 3 + dx) * 2 + kc
                lw5[:, o * 256:(o + 1) * 256] = w5[:, kc * 128:(kc + 1) * 128, dy, dx].T
    lb5 = np.stack([b5[0:128], b5[128:256]], axis=1).astype(f)
    return dict(lw1=lw1.astype(BF), lw2=lw2.astype(BF), lw2d6=lw2d6.astype(BF),
                lb2=lb2, lw3a=lw3a.astype(BF), lw3b=lw3b.astype(BF), lb3=lb3,
                lw4=lw4.astype(BF), lb4=lb4, lw5=lw5.astype(BF), lb5=lb5)


def _prep_x13(x):
    """x [B,3,32,32] -> per-core [8, 100, 4*41*32] im2col-packed bf16."""
    f = np.float32
    xpad = np.zeros((B, 3, 44, 42), f)
    xpad[:, :, 5:37, 5:37] = x
    X = np.zeros((100, B, 41, 32), f)
    for dyo in range(3):
        for dx in range(11):
            for c in range(3):
                X[dyo * 33 + dx * 3 + c] = xpad[:, c, dyo:dyo + 41, dx:dx + 32]
    X[99] = 1.0
    out = []
    for r in range(N_CORES):
        pc = X[:, r * BC:(r + 1) * BC]  # [100, 32, 41, 32]
        pc = pc.reshape(100, 8, 4 * 41 * 32).transpose(1, 0, 2)
        out.append(np.ascontiguousarray(pc.astype(BF)))
    return out


def _prep_fc(fw1, fb1, fw2, fb2, fw3, fb3):
    f = np.float32
    outs = []
    for r in range(N_CORES):
        sl = slice(512 * r, 512 * (r + 1))
        # fw1s [128, (yx, cc, m)]: fw1[512r+m, (cc*128+k)*16+yx]
        fw1s = fw1[sl].reshape(512, 2, 128, 16).transpose(2, 3, 1, 0).reshape(128, -1)
        fb1s = fb1[sl].reshape(4, 128).T
        # fw2s [128, (kc, m)]: fw2[512r+m, kc*128+k]
        fw2s = fw2[sl].reshape(512, 32, 128).transpose(2, 1, 0).reshape(128, -1)
        fb2s = fb2[sl].reshape(4, 128).T
        # fw3s [128, (kc, m)]: fw3[m, 512r + kc*128 + k]
        fw3s = fw3[:, sl].reshape(100, 4, 128).transpose(2, 1, 0).reshape(128, -1)
        fb3s = (fb3 / N_CORES).reshape(100, 1)
        outs.append(dict(
            fw1s=np.ascontiguousarray(fw1s.astype(BF)),
            fb1s=np.ascontiguousarray(fb1s.astype(f)),
            fw2s=np.ascontiguousarray(fw2s.astype(BF)),
            fb2s=np.ascontiguousarray(fb2s.astype(f)),
            fw3s=np.ascontiguousarray(fw3s.astype(BF)),
            fb3s=np.ascontiguousarray(fb3s.astype(f)),
        ))
    return outs


_CACHE = {}

_SHAPES = dict(
    x13=(8, 100, 4 * 41 * 32), lw1=(100, 4 * 128),
    lw2=(128, 7 * 3 * 256), lw2d6=(128, 4 * 256), lb2=(128, 2),
    lw3a=(128, 25 * 384), lw3b=(128, 13 * 384), lb3=(128, 3),
    lw4=(128, 27 * 256), lb4=(128, 2),
    lw5=(128, 18 * 256), lb5=(128, 2),
    fw1s=(128, 32 * 512), fb1s=(128, 4),
    fw2s=(128, 32 * 512), fb2s=(128, 4),
    fw3s=(128, 4 * 100), fb3s=(100, 1),
)

_BF16_INPUTS = {"x13", "lw1", "lw2", "lw2d6", "lw3a", "lw3b", "lw4", "lw5",
                "fw1s", "fw2s", "fw3s"}


def _build():
    if "nc" in _CACHE:
        return _CACHE["nc"]
    nc = bacc.Bacc("TRN2", target_bir_lowering=False, debug=False,
                   num_devices=N_CORES)
    t = {name: nc.dram_tensor(
            name, list(shape), BF16 if name in _BF16_INPUTS else F32,
            kind="ExternalInput").ap()
         for name, shape in _SHAPES.items()}
    yout = nc.dram_tensor("yout", [100, B], F32, kind="ExternalOutput").ap()
    with tile.TileContext(nc) as tc:
        _emit(nc, tc, t, yout)
    nc.compile()
    _CACHE["nc"] = nc
    return nc


def kernel(x, w1, b1, w2, b2, w3, b3, w4, b4, w5, b5,
           fw1, fb1, fw2, fb2, fw3, fb3):
    args = [np.asarray(a, np.float32) for a in
            (x, w1, b1, w2, b2, w3, b3, w4, b4, w5, b5, fw1, fb1, fw2, fb2, fw3, fb3)]
    (x, w1, b1, w2, b2, w3, b3, w4, b4, w5, b5,
     fw1, fb1, fw2, fb2, fw3, fb3) = args
    nc = _build()
    shared = _prep_shared(w1, b1, w2, b2, w3, b3, w4, b4, w5, b5)
    x13s = _prep_x13(x)
    fcs = _prep_fc(fw1, fb1, fw2, fb2, fw3, fb3)
    in_maps = [{**shared, "x13": x13s[r], **fcs[r]} for r in range(N_CORES)]
    res = run_bass_kernel_spmd(nc, in_maps, list(range(N_CORES)))
    y = res.results[0]["yout"]  # [100, 256]
    return np.ascontiguousarray(np.asarray(y, np.float32).T)
